# revision 40
# baseline (speedup 1.0000x reference)
"""Trainium2 Bass kernel for a DeepSeek-style MoE block (full-I/O contract).

Strategy (8 NeuronCores):
  - Expert-parallel: E=16 routed experts, 2 per core. Host computes the gate
    (softmax + top-4) in numpy, gathers each expert's tokens, and ships
    transposed token blocks per core. Experts are ranked by token count:
    ranks 0-7 go to slot 0 (capacity C0), ranks 8-15 to slot 1 (C1 <= C0),
    so padding waste tracks the actual load distribution.
  - Routed experts run in fp8 (e4m3) with DoubleRow matmuls (2x PE
    throughput): weights are host-scaled into e4m3 range (w_gate x32,
    w_up x8, w_down x64), activations quantized on the fly; the silu
    applies 1/32 as its input scale and the routing weights fold the
    remaining 1/(8*64) at PSUM eviction. Accumulation stays fp32.
  - Shared expert stays bf16 (it dominates the output norm; fp8 there
    would blow the error budget) and is split 2D: core c computes token
    block c//2 (512 tokens) x F-half c%2 (1408 of Fs=2816). That shape is
    SPMD-uniform, has zero padding, and reuses the routed-expert loop.
  - Host scatter-adds the routed partials and sums the shared partials.
"""

import math
from contextlib import ExitStack

import ml_dtypes
import numpy as np

T = 2048
H = 2048
E = 16
TOP_K = 4
F = 1408
FS = 2816
N_CORES = 8
EPC = E // N_CORES  # experts per core = 2
KH = H // 128  # 16 contraction chunks over H
KH2 = KH // 2  # 8 DoubleRow pairs
NF = F // 128  # 11 F tiles (also the shared F-half tile count)
NF2 = NF // 2  # 5 DoubleRow pairs (+1 single tail chunk)
NH = H // 512  # 4 output H tiles
CS = T // (N_CORES // 2)  # 512-token shared block per core pair

BF16 = ml_dtypes.bfloat16
E4 = ml_dtypes.float8_e4m3

SWG = 32.0  # w_gate fp8 scale
SWU = 8.0   # w_up fp8 scale (also the a_q scale; keeps |a_q| < 100 << 240)
SD = 64.0   # w_down fp8 scale
WTS_DIV = SWU * SD  # folded into routing weights at eviction

_BUILD_CACHE: dict[tuple, object] = {}
last_exec_time_ns = None


def _routing(x: np.ndarray, gate_weight: np.ndarray):
    """Replicates the reference gate: fp32 logits, softmax, top-4 (ties ->
    lower expert index, matching jax.lax.top_k), no renorm."""
    logits = x.astype(np.float32) @ gate_weight.astype(np.float32).T
    z = logits - logits.max(axis=1, keepdims=True)
    p = np.exp(z)
    p /= p.sum(axis=1, keepdims=True)
    top_idx = np.argsort(-p, axis=1, kind="stable")[:, :TOP_K]
    top_vals = np.take_along_axis(p, top_idx, axis=1).astype(np.float32)
    return top_idx, top_vals


def _chunks(C):
    n = max(1, math.ceil(C / 512))
    while C % n:
        n += 1
    return C // n


def _build(caps: tuple):
    """Build + compile the SPMD one-core Bass graph for slot capacities."""
    key = tuple(caps)
    if key in _BUILD_CACHE:
        return _BUILD_CACHE[key]

    import concourse.bass as bass  # noqa: F401
    from concourse import bacc, mybir, tile

    bf = mybir.dt.bfloat16
    f32 = mybir.dt.float32
    fp8 = mybir.dt.float8e4
    DR = mybir.MatmulPerfMode.DoubleRow
    Silu = mybir.ActivationFunctionType.Silu

    CTs = [(C + 127) // 128 for C in caps]
    CTsum = sum(CTs)
    off = [0, caps[0]]  # row offsets into rout

    nc = bacc.Bacc(None, target_bir_lowering=False)

    xg_ds = [
        nc.dram_tensor(f"xg{e}", [128, KH, caps[e]], fp8, kind="ExternalInput")
        for e in range(EPC)
    ]
    wg_d = nc.dram_tensor("wg", [EPC, NF, 128, KH, 128], fp8, kind="ExternalInput")
    wu_d = nc.dram_tensor("wu", [EPC, NF, 128, KH, 128], fp8, kind="ExternalInput")
    wd_d = nc.dram_tensor("wd", [EPC, NH, 128, NF, 512], fp8, kind="ExternalInput")
    xs_d = nc.dram_tensor("xs", [128, KH, CS], bf, kind="ExternalInput")
    wsg_d = nc.dram_tensor("wsg", [NF, 128, KH, 128], bf, kind="ExternalInput")
    wsu_d = nc.dram_tensor("wsu", [NF, 128, KH, 128], bf, kind="ExternalInput")
    wsd_d = nc.dram_tensor("wsd", [NH, 128, NF, 512], bf, kind="ExternalInput")
    wts_d = nc.dram_tensor("wts", [128, CTsum], f32, kind="ExternalInput")
    rout = nc.dram_tensor("rout", [sum(caps), H], bf, kind="ExternalOutput")
    sout = nc.dram_tensor("sout", [CS, H], bf, kind="ExternalOutput")

    with tile.TileContext(nc) as tc, ExitStack() as ctx:
        const = ctx.enter_context(tc.tile_pool(name="const", bufs=1))
        bias0 = const.tile([128, 1], f32)
        nc.vector.memset(bias0[:], 0.0)
        wts_t = const.tile([128, CTsum], f32)
        nc.sync.dma_start(wts_t[:], wts_d[:])

        x_pool = ctx.enter_context(tc.tile_pool(name="xp", bufs=1))
        wgu_pool = ctx.enter_context(tc.tile_pool(name="wgu", bufs=4))
        wd_pool = ctx.enter_context(tc.tile_pool(name="wdp", bufs=4))
        a_pool = ctx.enter_context(tc.tile_pool(name="atp", bufs=1))
        ev_pool = ctx.enter_context(tc.tile_pool(name="evp", bufs=6))
        sg_pool = ctx.enter_context(tc.tile_pool(name="sgp", bufs=4))
        psum_gu = ctx.enter_context(tc.tile_pool(name="pgu", bufs=2, space="PSUM"))
        psum_d = ctx.enter_context(tc.tile_pool(name="pdp", bufs=4, space="PSUM"))

        # A DMA queue streams ~20-50 GB/s and serves FIFO, so (a) every
        # transfer is kept <= ~500 KB so round-robin interleaves streams,
        # and (b) pieces are emitted in consumption order — prefetching too
        # early blocks later-critical transfers behind them in the queue.
        x_tiles = {}

        def load_x(tag, C, dt, x_d, npc=4):
            x_t = x_pool.tile([128, KH, C], dt, tag=f"x{tag}", name=f"x{tag}")
            w = KH // npc
            for q in range(npc):
                nc.sync.dma_start(
                    x_t[:, q * w:(q + 1) * w], x_d[:, q * w:(q + 1) * w]
                )
            x_tiles[tag] = x_t

        load_x("r0", caps[0], fp8, xg_ds[0])

        # HAM warmup: keep the PE busy during the initial DMA wait so the
        # clock-gate is at 8/8 when the first real matmuls arrive
        warm = const.tile([128, 512], bf, name="warm")
        nc.vector.memset(warm[:], 0.0)
        warmout = const.tile([128, 1], f32, name="warmout")
        wpsum = psum_gu.tile([128, 512], f32, tag="pg", name="warmp")
        for _ in range(20):
            nc.tensor.matmul(wpsum, warm[:, :128], warm[:], start=True, stop=True)
        nc.vector.tensor_copy(out=warmout[:], in_=wpsum[:, :1])

        def expert(tag, C, is_fp8, wg_e, wu_e, wd_e, out_d, out_off, wts_col,
                   first=False, last=False, prefetch=None):
            """One gated-MLP expert: gate/up -> silu*mul -> down.

            fp8 experts run DoubleRow matmuls and apply the routing weight
            (pre-divided by SWU*SD) at eviction; the bf16 shared expert
            evicts with a plain copy. `prefetch` (next expert's x load) is
            emitted mid-way through gate/up: late enough not to block this
            expert's weight stream, early enough to land before it's needed."""
            dt = fp8 if is_fp8 else bf
            CT = (C + 127) // 128
            cw = _chunks(C)
            nch = C // cw
            x_t = x_tiles[tag]
            aT = a_pool.tile([128, NF, C], dt, tag=f"aT{tag}", name=f"aT{tag}")
            wd_ts = []
            for f in range(NF):
                wg_t = wgu_pool.tile([128, KH, 128], dt, tag=f"wg{is_fp8}")
                wu_t = wgu_pool.tile([128, KH, 128], dt, tag=f"wu{is_fp8}")
                if (first and f == 0) or not is_fp8:
                    nc.sync.dma_start(wg_t[:, :8], wg_e[f, :, :8])
                    nc.sync.dma_start(wg_t[:, 8:], wg_e[f, :, 8:])
                    nc.sync.dma_start(wu_t[:, :8], wu_e[f, :, :8])
                    nc.sync.dma_start(wu_t[:, 8:], wu_e[f, :, 8:])
                else:
                    nc.sync.dma_start(wg_t[:], wg_e[f])
                    nc.sync.dma_start(wu_t[:], wu_e[f])
                if 2 <= f <= 5:
                    # preload one down-proj tile per F-iteration (needed right
                    # after the last gate/up group); spreading them keeps the
                    # burst from delaying this expert's own weight stream
                    h = f - 2
                    npc = 2 if is_fp8 else 4  # pieces ~360 KB
                    bnd = [round(i * NF / npc) for i in range(npc + 1)]
                    wd_t = wd_pool.tile([128, NF, 512], dt, tag=f"wd{is_fp8}")
                    for i in range(npc):
                        nc.sync.dma_start(
                            wd_t[:, bnd[i]:bnd[i + 1]],
                            wd_e[h, :, bnd[i]:bnd[i + 1]],
                        )
                    wd_ts.append(wd_t)
                if f == 5 and prefetch is not None:
                    prefetch()
                for j in range(nch):
                    csl = slice(j * cw, (j + 1) * cw)
                    pg = psum_gu.tile([128, 512], f32, tag="pg", name="pg")[:, :cw]
                    pu = psum_gu.tile([128, 512], f32, tag="pu", name="pu")[:, :cw]
                    if is_fp8:
                        for k in range(KH2):
                            nc.tensor.matmul(
                                pg, wg_t[:, 2 * k:2 * k + 2],
                                x_t[:, 2 * k:2 * k + 2, csl],
                                start=(k == 0), stop=(k == KH2 - 1), perf_mode=DR,
                            )
                        for k in range(KH2):
                            nc.tensor.matmul(
                                pu, wu_t[:, 2 * k:2 * k + 2],
                                x_t[:, 2 * k:2 * k + 2, csl],
                                start=(k == 0), stop=(k == KH2 - 1), perf_mode=DR,
                            )
                    else:
                        for k in range(KH):
                            nc.tensor.matmul(
                                pg, wg_t[:, k], x_t[:, k, csl],
                                start=(k == 0), stop=(k == KH - 1),
                            )
                        for k in range(KH):
                            nc.tensor.matmul(
                                pu, wu_t[:, k], x_t[:, k, csl],
                                start=(k == 0), stop=(k == KH - 1),
                            )
                    sg = sg_pool.tile([128, 512], f32, tag="sg", name="sg")[:, :cw]
                    # fp8: pg holds 32*g -> silu(g); pu holds 8*u, so the
                    # e4m3 store of sg*pu is a_q = e4m3(8*a)
                    nc.scalar.activation(
                        sg, pg, Silu, bias=bias0[:],
                        scale=(1.0 / SWG) if is_fp8 else 1.0,
                    )
                    nc.vector.tensor_mul(aT[:, f, csl], sg, pu)
            for h in range(NH):
                wd_t = wd_ts[h]
                for ct in range(CT):
                    tw = min(128, C - ct * 128)
                    pd = psum_d.tile([128, 512], f32, tag="pd", name="pd")[:tw]
                    if is_fp8:
                        for fo in range(NF2):
                            nc.tensor.matmul(
                                pd,
                                aT[:, 2 * fo:2 * fo + 2, ct * 128: ct * 128 + tw],
                                wd_t[:, 2 * fo:2 * fo + 2],
                                start=(fo == 0), stop=False, perf_mode=DR,
                            )
                        nc.tensor.matmul(
                            pd, aT[:, NF - 1, ct * 128: ct * 128 + tw],
                            wd_t[:, NF - 1],
                            start=False, stop=True,
                        )
                    else:
                        for fo in range(NF):
                            nc.tensor.matmul(
                                pd, aT[:, fo, ct * 128: ct * 128 + tw],
                                wd_t[:, fo],
                                start=(fo == 0), stop=(fo == NF - 1),
                            )
                    ob = ev_pool.tile([128, 512], bf, tag="ob", name="ob")[:tw]
                    if wts_col is not None:
                        nc.vector.tensor_scalar_mul(
                            ob[:], pd[:], wts_t[:tw, wts_col + ct:wts_col + ct + 1]
                        )
                    else:
                        nc.any.tensor_copy(out=ob[:], in_=pd[:])
                    rsl = slice(out_off + ct * 128, out_off + ct * 128 + tw)
                    if last and h == NH - 1:
                        # the kernel's tail is the final strip DMA (~6us for
                        # 128KB on one ~21GB/s queue): quarter it across queues
                        for qq in range(4):
                            nc.sync.dma_start(
                                out_d[rsl, h * 512 + qq * 128:
                                      h * 512 + (qq + 1) * 128],
                                ob[:, qq * 128:(qq + 1) * 128],
                            )
                    else:
                        nc.sync.dma_start(
                            out_d[rsl, h * 512:(h + 1) * 512], ob[:]
                        )

        # slot-0 routed expert first: its fp8 token block is the smallest
        # initial DMA, so real matmuls start soonest after warmup
        expert("r0", caps[0], True, wg_d[0], wu_d[0], wd_d[0],
               rout, off[0], 0, first=True,
               prefetch=lambda: load_x("s", CS, bf, xs_d))
        expert("s", CS, False, wsg_d, wsu_d, wsd_d, sout, 0, None,
               prefetch=lambda: load_x("r1", caps[1], fp8, xg_ds[1]))
        expert("r1", caps[1], True, wg_d[1], wu_d[1], wd_d[1],
               rout, off[1], CTs[0], last=True)

    nc.compile()
    _BUILD_CACHE[key] = nc
    return nc


def kernel(**inputs: np.ndarray) -> np.ndarray:
    global last_exec_time_ns
    from concourse.bass_utils import run_bass_kernel_spmd

    hs = inputs["hidden_states"]
    x = np.ascontiguousarray(hs.reshape(T, H), dtype=np.float32)
    top_idx, top_vals = _routing(x, inputs["gate_weight"])

    # per-expert token lists (ascending token order)
    rows_per_e = []
    for e in range(E):
        rows, kpos = np.nonzero(top_idx == e)
        rows_per_e.append((rows, top_vals[rows, kpos]))
    counts = np.array([len(r) for r, _ in rows_per_e])
    # rank experts by load: ranks 0..7 -> slot 0 of cores 0..7 (big slots),
    # ranks 8..15 -> slot 1 of cores 7..0 (small slots)
    order = np.argsort(-counts, kind="stable")
    slot_expert = np.empty((N_CORES, EPC), np.int64)
    for i in range(N_CORES):
        slot_expert[i, 0] = order[i]
        slot_expert[i, 1] = order[E - 1 - i]
    cap = lambda n: max(128, ((n + 7) // 8) * 8)
    caps = (
        cap(int(counts[slot_expert[:, 0]].max())),
        cap(int(counts[slot_expert[:, 1]].max())),
    )
    CTs = [(C + 127) // 128 for C in caps]
    CTsum = sum(CTs)

    nc = _build(caps)

    xb = x.astype(BF16)
    # per-block shared tokens [128, KH, 512]: xs[p, k, t'] = x[b*512+t', k*128+p]
    xsR = [
        np.ascontiguousarray(
            xb[b * CS:(b + 1) * CS].reshape(CS, KH, 128).transpose(2, 1, 0)
        )
        for b in range(N_CORES // 2)
    ]

    w_gate = inputs["w_gate"]
    w_up = inputs["w_up"]
    w_down = inputs["w_down"]
    ws_gate = inputs["ws_gate"].astype(BF16)
    ws_up = inputs["ws_up"].astype(BF16)
    ws_down = inputs["ws_down"].astype(BF16)
    # per-F-half shared weight layouts
    wsgR = [
        np.ascontiguousarray(
            ws_gate[:, half * F:(half + 1) * F]
            .reshape(KH, 128, NF, 128).transpose(2, 1, 0, 3)
        )
        for half in range(2)
    ]
    wsuR = [
        np.ascontiguousarray(
            ws_up[:, half * F:(half + 1) * F]
            .reshape(KH, 128, NF, 128).transpose(2, 1, 0, 3)
        )
        for half in range(2)
    ]
    wsdR = [
        np.ascontiguousarray(
            ws_down[half * F:(half + 1) * F]
            .reshape(NF, 128, NH, 512).transpose(2, 1, 0, 3)
        )
        for half in range(2)
    ]

    in_maps = []
    for c in range(N_CORES):
        wtsR = np.zeros((128, CTsum), np.float32)
        wgR = np.empty((EPC, NF, 128, KH, 128), E4)
        wuR = np.empty((EPC, NF, 128, KH, 128), E4)
        wdR = np.empty((EPC, NH, 128, NF, 512), E4)
        imap = {}
        for el in range(EPC):
            C = caps[el]
            CT = CTs[el]
            ge = int(slot_expert[c, el])
            rows, wts = rows_per_e[ge]
            n = len(rows)
            xgR = np.zeros((128, KH, C), E4)
            if n:
                # [n, H] -> [128, KH, n], f32 -> e4m3 directly
                xgR[:, :, :n] = (
                    x[rows].astype(E4).reshape(n, KH, 128).transpose(2, 1, 0)
                )
                wcol = np.zeros(CT * 128, np.float32)
                wcol[:n] = wts / WTS_DIV
                base = sum(CTs[:el])
                wtsR[:, base:base + CT] = wcol.reshape(CT, 128).T
            imap[f"xg{el}"] = xgR
            wgR[el] = (
                (SWG * w_gate[ge]).astype(E4)
                .reshape(KH, 128, NF, 128).transpose(2, 1, 0, 3)
            )
            wuR[el] = (
                (SWU * w_up[ge]).astype(E4)
                .reshape(KH, 128, NF, 128).transpose(2, 1, 0, 3)
            )
            wdR[el] = (
                (SD * w_down[ge]).astype(E4)
                .reshape(NF, 128, NH, 512).transpose(2, 1, 0, 3)
            )
        imap.update(
            wg=wgR,
            wu=wuR,
            wd=wdR,
            xs=xsR[c // 2],
            wsg=wsgR[c % 2],
            wsu=wsuR[c % 2],
            wsd=wsdR[c % 2],
            wts=wtsR,
        )
        in_maps.append(imap)

    res = run_bass_kernel_spmd(nc, in_maps, core_ids=list(range(N_CORES)))
    last_exec_time_ns = res.exec_time_ns

    out = np.zeros((T, H), np.float32)
    off = [0, caps[0]]
    for c in range(N_CORES):
        r = res.results[c]
        b = c // 2
        out[b * CS:(b + 1) * CS] += r["sout"].astype(np.float32)
        for el in range(EPC):
            rows, _ = rows_per_e[int(slot_expert[c, el])]
            n = len(rows)
            if n:
                # rows are unique within one expert, so fancy-index add is safe
                out[rows] += r["rout"][off[el]: off[el] + n].astype(np.float32)
    return out.reshape(hs.shape).astype(hs.dtype)


# revision 43
# speedup vs baseline: 1.0221x; 1.0221x over previous
"""Trainium2 Bass kernel for a DeepSeek-style MoE block (full-I/O contract).

Strategy (8 NeuronCores):
  - Expert-parallel: E=16 routed experts, 2 per core. Host computes the gate
    (softmax + top-4) in numpy, gathers each expert's tokens, and ships
    transposed token blocks per core. Experts are ranked by token count:
    ranks 0-7 go to slot 0 (capacity C0), ranks 8-15 to slot 1 (C1 <= C0),
    so padding waste tracks the actual load distribution.
  - Routed experts run in fp8 (e4m3) with DoubleRow matmuls (2x PE
    throughput): weights are host-scaled into e4m3 range (w_gate x32,
    w_up x8, w_down x64), activations quantized on the fly; the silu
    applies 1/32 as its input scale and the routing weights fold the
    remaining 1/(8*64) at PSUM eviction. Accumulation stays fp32.
  - Shared expert stays bf16 (it dominates the output norm; fp8 there
    would blow the error budget) and is split 2D: core c computes token
    block c//2 (512 tokens) x F-half c%2 (1408 of Fs=2816). That shape is
    SPMD-uniform, has zero padding, and reuses the routed-expert loop.
  - Host scatter-adds the routed partials and sums the shared partials.
"""

import math
from contextlib import ExitStack

import ml_dtypes
import numpy as np

T = 2048
H = 2048
E = 16
TOP_K = 4
F = 1408
FS = 2816
N_CORES = 8
EPC = E // N_CORES  # experts per core = 2
KH = H // 128  # 16 contraction chunks over H
KH2 = KH // 2  # 8 DoubleRow pairs
NF = F // 128  # 11 F tiles (also the shared F-half tile count)
NF2 = NF // 2  # 5 DoubleRow pairs (+1 single tail chunk)
NH = H // 512  # 4 output H tiles
CS = T // (N_CORES // 2)  # 512-token shared block per core pair

BF16 = ml_dtypes.bfloat16
E4 = ml_dtypes.float8_e4m3

SWG = 32.0  # w_gate fp8 scale
SWU = 8.0   # w_up fp8 scale (also the a_q scale; keeps |a_q| < 100 << 240)
SD = 64.0   # w_down fp8 scale
WTS_DIV = SWU * SD  # folded into routing weights at eviction

_BUILD_CACHE: dict[tuple, object] = {}
last_exec_time_ns = None


def _routing(x: np.ndarray, gate_weight: np.ndarray):
    """Replicates the reference gate: fp32 logits, softmax, top-4 (ties ->
    lower expert index, matching jax.lax.top_k), no renorm."""
    logits = x.astype(np.float32) @ gate_weight.astype(np.float32).T
    z = logits - logits.max(axis=1, keepdims=True)
    p = np.exp(z)
    p /= p.sum(axis=1, keepdims=True)
    top_idx = np.argsort(-p, axis=1, kind="stable")[:, :TOP_K]
    top_vals = np.take_along_axis(p, top_idx, axis=1).astype(np.float32)
    return top_idx, top_vals


def _chunks(C):
    n = max(1, math.ceil(C / 512))
    while C % n:
        n += 1
    return C // n


def _build(caps: tuple):
    """Build + compile the SPMD one-core Bass graph for slot capacities."""
    key = tuple(caps)
    if key in _BUILD_CACHE:
        return _BUILD_CACHE[key]

    import concourse.bass as bass  # noqa: F401
    from concourse import bacc, mybir, tile

    bf = mybir.dt.bfloat16
    f32 = mybir.dt.float32
    fp8 = mybir.dt.float8e4
    DR = mybir.MatmulPerfMode.DoubleRow
    Silu = mybir.ActivationFunctionType.Silu

    CTs = [(C + 127) // 128 for C in caps]
    CTsum = sum(CTs)
    off = [0, caps[0]]  # row offsets into rout

    nc = bacc.Bacc(None, target_bir_lowering=False)

    xg_ds = [
        nc.dram_tensor(f"xg{e}", [128, KH, caps[e]], fp8, kind="ExternalInput")
        for e in range(EPC)
    ]
    wg_d = nc.dram_tensor("wg", [EPC, NF, 128, KH, 128], fp8, kind="ExternalInput")
    wu_d = nc.dram_tensor("wu", [EPC, NF, 128, KH, 128], fp8, kind="ExternalInput")
    wd_d = nc.dram_tensor("wd", [EPC, NH, 128, NF, 512], fp8, kind="ExternalInput")
    xs_d = nc.dram_tensor("xs", [128, KH, CS], bf, kind="ExternalInput")
    wsg_d = nc.dram_tensor("wsg", [NF, 128, KH, 128], bf, kind="ExternalInput")
    wsu_d = nc.dram_tensor("wsu", [NF, 128, KH, 128], bf, kind="ExternalInput")
    wsd_d = nc.dram_tensor("wsd", [NH, 128, NF, 512], bf, kind="ExternalInput")
    wts_d = nc.dram_tensor("wts", [128, CTsum], f32, kind="ExternalInput")
    rout = nc.dram_tensor("rout", [sum(caps), H], bf, kind="ExternalOutput")
    sout = nc.dram_tensor("sout", [CS, H], bf, kind="ExternalOutput")

    with tile.TileContext(nc) as tc, ExitStack() as ctx:
        const = ctx.enter_context(tc.tile_pool(name="const", bufs=1))
        bias0 = const.tile([128, 1], f32)
        nc.vector.memset(bias0[:], 0.0)
        wts_t = const.tile([128, CTsum], f32)
        nc.sync.dma_start(wts_t[:], wts_d[:])

        x_pool = ctx.enter_context(tc.tile_pool(name="xp", bufs=1))
        wgu_pool = ctx.enter_context(tc.tile_pool(name="wgu", bufs=4))
        wd_pool = ctx.enter_context(tc.tile_pool(name="wdp", bufs=4))
        a_pool = ctx.enter_context(tc.tile_pool(name="atp", bufs=1))
        ev_pool = ctx.enter_context(tc.tile_pool(name="evp", bufs=6))
        sg_pool = ctx.enter_context(tc.tile_pool(name="sgp", bufs=4))
        psum_gu = ctx.enter_context(tc.tile_pool(name="pgu", bufs=2, space="PSUM"))
        psum_d = ctx.enter_context(tc.tile_pool(name="pdp", bufs=4, space="PSUM"))

        # A DMA queue streams ~20-50 GB/s and serves FIFO, so (a) every
        # transfer is kept <= ~500 KB so round-robin interleaves streams,
        # and (b) pieces are emitted in consumption order — prefetching too
        # early blocks later-critical transfers behind them in the queue.
        x_tiles = {}

        def load_x(tag, C, dt, x_d, npc=4):
            x_t = x_pool.tile([128, KH, C], dt, tag=f"x{tag}", name=f"x{tag}")
            w = KH // npc
            for q in range(npc):
                nc.sync.dma_start(
                    x_t[:, q * w:(q + 1) * w], x_d[:, q * w:(q + 1) * w]
                )
            x_tiles[tag] = x_t

        load_x("r0", caps[0], fp8, xg_ds[0])

        # HAM warmup: keep the PE busy during the initial DMA wait so the
        # clock-gate is at 8/8 when the first real matmuls arrive
        warm = const.tile([128, 512], bf, name="warm")
        nc.vector.memset(warm[:], 0.0)
        warmout = const.tile([128, 1], f32, name="warmout")
        wpsum = psum_gu.tile([128, 512], f32, tag="pg", name="warmp")
        for _ in range(20):
            nc.tensor.matmul(wpsum, warm[:, :128], warm[:], start=True, stop=True)
        nc.vector.tensor_copy(out=warmout[:], in_=wpsum[:, :1])

        def expert(tag, C, is_fp8, wg_e, wu_e, wd_e, out_d, out_off, wts_col,
                   first=False, prefetch=None):
            """One gated-MLP expert: gate/up -> silu*mul -> down.

            fp8 experts run DoubleRow matmuls and apply the routing weight
            (pre-divided by SWU*SD) at eviction; the bf16 shared expert
            evicts with a plain copy. `prefetch` (next expert's x load) is
            emitted mid-way through gate/up: late enough not to block this
            expert's weight stream, early enough to land before it's needed."""
            dt = fp8 if is_fp8 else bf
            CT = (C + 127) // 128
            cw = _chunks(C)
            nch = C // cw
            x_t = x_tiles[tag]
            aT = a_pool.tile([128, NF, C], dt, tag=f"aT{tag}", name=f"aT{tag}")
            wd_ts = []
            for f in range(NF):
                wg_t = wgu_pool.tile([128, KH, 128], dt, tag=f"wg{is_fp8}")
                wu_t = wgu_pool.tile([128, KH, 128], dt, tag=f"wu{is_fp8}")
                if (first and f == 0) or not is_fp8:
                    nc.sync.dma_start(wg_t[:, :8], wg_e[f, :, :8])
                    nc.sync.dma_start(wg_t[:, 8:], wg_e[f, :, 8:])
                    nc.sync.dma_start(wu_t[:, :8], wu_e[f, :, :8])
                    nc.sync.dma_start(wu_t[:, 8:], wu_e[f, :, 8:])
                else:
                    nc.sync.dma_start(wg_t[:], wg_e[f])
                    nc.sync.dma_start(wu_t[:], wu_e[f])
                if 2 <= f <= 5:
                    # preload one down-proj tile per F-iteration (needed right
                    # after the last gate/up group); spreading them keeps the
                    # burst from delaying this expert's own weight stream
                    h = f - 2
                    npc = 2 if is_fp8 else 4  # pieces ~360 KB
                    bnd = [round(i * NF / npc) for i in range(npc + 1)]
                    wd_t = wd_pool.tile([128, NF, 512], dt, tag=f"wd{is_fp8}")
                    for i in range(npc):
                        nc.sync.dma_start(
                            wd_t[:, bnd[i]:bnd[i + 1]],
                            wd_e[h, :, bnd[i]:bnd[i + 1]],
                        )
                    wd_ts.append(wd_t)
                if f == 5 and prefetch is not None:
                    prefetch()
                for j in range(nch):
                    csl = slice(j * cw, (j + 1) * cw)
                    pg = psum_gu.tile([128, 512], f32, tag="pg", name="pg")[:, :cw]
                    pu = psum_gu.tile([128, 512], f32, tag="pu", name="pu")[:, :cw]
                    if is_fp8:
                        for k in range(KH2):
                            nc.tensor.matmul(
                                pg, wg_t[:, 2 * k:2 * k + 2],
                                x_t[:, 2 * k:2 * k + 2, csl],
                                start=(k == 0), stop=(k == KH2 - 1), perf_mode=DR,
                            )
                        for k in range(KH2):
                            nc.tensor.matmul(
                                pu, wu_t[:, 2 * k:2 * k + 2],
                                x_t[:, 2 * k:2 * k + 2, csl],
                                start=(k == 0), stop=(k == KH2 - 1), perf_mode=DR,
                            )
                    else:
                        for k in range(KH):
                            nc.tensor.matmul(
                                pg, wg_t[:, k], x_t[:, k, csl],
                                start=(k == 0), stop=(k == KH - 1),
                            )
                        for k in range(KH):
                            nc.tensor.matmul(
                                pu, wu_t[:, k], x_t[:, k, csl],
                                start=(k == 0), stop=(k == KH - 1),
                            )
                    sg = sg_pool.tile([128, 512], f32, tag="sg", name="sg")[:, :cw]
                    # fp8: pg holds 32*g -> silu(g); pu holds 8*u, so the
                    # e4m3 store of sg*pu is a_q = e4m3(8*a)
                    nc.scalar.activation(
                        sg, pg, Silu, bias=bias0[:],
                        scale=(1.0 / SWG) if is_fp8 else 1.0,
                    )
                    nc.vector.tensor_mul(aT[:, f, csl], sg, pu)
            for h in range(NH):
                wd_t = wd_ts[h]
                for ct in range(CT):
                    tw = min(128, C - ct * 128)
                    pd = psum_d.tile([128, 512], f32, tag="pd", name="pd")[:tw]
                    if is_fp8:
                        for fo in range(NF2):
                            nc.tensor.matmul(
                                pd,
                                aT[:, 2 * fo:2 * fo + 2, ct * 128: ct * 128 + tw],
                                wd_t[:, 2 * fo:2 * fo + 2],
                                start=(fo == 0), stop=False, perf_mode=DR,
                            )
                        nc.tensor.matmul(
                            pd, aT[:, NF - 1, ct * 128: ct * 128 + tw],
                            wd_t[:, NF - 1],
                            start=False, stop=True,
                        )
                    else:
                        for fo in range(NF):
                            nc.tensor.matmul(
                                pd, aT[:, fo, ct * 128: ct * 128 + tw],
                                wd_t[:, fo],
                                start=(fo == 0), stop=(fo == NF - 1),
                            )
                    ob = ev_pool.tile([128, 512], bf, tag="ob", name="ob")[:tw]
                    if wts_col is not None:
                        nc.vector.tensor_scalar_mul(
                            ob[:], pd[:], wts_t[:tw, wts_col + ct:wts_col + ct + 1]
                        )
                    else:
                        nc.any.tensor_copy(out=ob[:], in_=pd[:])
                    # one full 512-wide strip per DMA: narrower writes have
                    # sub-1KB partition lines and drain slower, not faster
                    nc.sync.dma_start(
                        out_d[out_off + ct * 128: out_off + ct * 128 + tw,
                              h * 512:(h + 1) * 512],
                        ob[:],
                    )

        # slot-0 routed expert first: its fp8 token block is the smallest
        # initial DMA, so real matmuls start soonest after warmup
        expert("r0", caps[0], True, wg_d[0], wu_d[0], wd_d[0],
               rout, off[0], 0, first=True,
               prefetch=lambda: load_x("s", CS, bf, xs_d))
        expert("s", CS, False, wsg_d, wsu_d, wsd_d, sout, 0, None,
               prefetch=lambda: load_x("r1", caps[1], fp8, xg_ds[1]))
        expert("r1", caps[1], True, wg_d[1], wu_d[1], wd_d[1],
               rout, off[1], CTs[0])

    nc.compile()
    _BUILD_CACHE[key] = nc
    return nc


def kernel(**inputs: np.ndarray) -> np.ndarray:
    global last_exec_time_ns
    from concourse.bass_utils import run_bass_kernel_spmd

    hs = inputs["hidden_states"]
    x = np.ascontiguousarray(hs.reshape(T, H), dtype=np.float32)
    top_idx, top_vals = _routing(x, inputs["gate_weight"])

    # per-expert token lists (ascending token order)
    rows_per_e = []
    for e in range(E):
        rows, kpos = np.nonzero(top_idx == e)
        rows_per_e.append((rows, top_vals[rows, kpos]))
    counts = np.array([len(r) for r, _ in rows_per_e])
    # rank experts by load: ranks 0..7 -> slot 0 of cores 0..7 (big slots),
    # ranks 8..15 -> slot 1 of cores 7..0 (small slots)
    order = np.argsort(-counts, kind="stable")
    slot_expert = np.empty((N_CORES, EPC), np.int64)
    for i in range(N_CORES):
        slot_expert[i, 0] = order[i]
        slot_expert[i, 1] = order[E - 1 - i]
    cap = lambda n: max(128, ((n + 7) // 8) * 8)
    caps = (
        cap(int(counts[slot_expert[:, 0]].max())),
        cap(int(counts[slot_expert[:, 1]].max())),
    )
    CTs = [(C + 127) // 128 for C in caps]
    CTsum = sum(CTs)

    nc = _build(caps)

    xb = x.astype(BF16)
    # per-block shared tokens [128, KH, 512]: xs[p, k, t'] = x[b*512+t', k*128+p]
    xsR = [
        np.ascontiguousarray(
            xb[b * CS:(b + 1) * CS].reshape(CS, KH, 128).transpose(2, 1, 0)
        )
        for b in range(N_CORES // 2)
    ]

    w_gate = inputs["w_gate"]
    w_up = inputs["w_up"]
    w_down = inputs["w_down"]
    ws_gate = inputs["ws_gate"].astype(BF16)
    ws_up = inputs["ws_up"].astype(BF16)
    ws_down = inputs["ws_down"].astype(BF16)
    # per-F-half shared weight layouts
    wsgR = [
        np.ascontiguousarray(
            ws_gate[:, half * F:(half + 1) * F]
            .reshape(KH, 128, NF, 128).transpose(2, 1, 0, 3)
        )
        for half in range(2)
    ]
    wsuR = [
        np.ascontiguousarray(
            ws_up[:, half * F:(half + 1) * F]
            .reshape(KH, 128, NF, 128).transpose(2, 1, 0, 3)
        )
        for half in range(2)
    ]
    wsdR = [
        np.ascontiguousarray(
            ws_down[half * F:(half + 1) * F]
            .reshape(NF, 128, NH, 512).transpose(2, 1, 0, 3)
        )
        for half in range(2)
    ]

    in_maps = []
    for c in range(N_CORES):
        wtsR = np.zeros((128, CTsum), np.float32)
        wgR = np.empty((EPC, NF, 128, KH, 128), E4)
        wuR = np.empty((EPC, NF, 128, KH, 128), E4)
        wdR = np.empty((EPC, NH, 128, NF, 512), E4)
        imap = {}
        for el in range(EPC):
            C = caps[el]
            CT = CTs[el]
            ge = int(slot_expert[c, el])
            rows, wts = rows_per_e[ge]
            n = len(rows)
            xgR = np.zeros((128, KH, C), E4)
            if n:
                # [n, H] -> [128, KH, n], f32 -> e4m3 directly
                xgR[:, :, :n] = (
                    x[rows].astype(E4).reshape(n, KH, 128).transpose(2, 1, 0)
                )
                wcol = np.zeros(CT * 128, np.float32)
                wcol[:n] = wts / WTS_DIV
                base = sum(CTs[:el])
                wtsR[:, base:base + CT] = wcol.reshape(CT, 128).T
            imap[f"xg{el}"] = xgR
            wgR[el] = (
                (SWG * w_gate[ge]).astype(E4)
                .reshape(KH, 128, NF, 128).transpose(2, 1, 0, 3)
            )
            wuR[el] = (
                (SWU * w_up[ge]).astype(E4)
                .reshape(KH, 128, NF, 128).transpose(2, 1, 0, 3)
            )
            wdR[el] = (
                (SD * w_down[ge]).astype(E4)
                .reshape(NF, 128, NH, 512).transpose(2, 1, 0, 3)
            )
        imap.update(
            wg=wgR,
            wu=wuR,
            wd=wdR,
            xs=xsR[c // 2],
            wsg=wsgR[c % 2],
            wsu=wsuR[c % 2],
            wsd=wsdR[c % 2],
            wts=wtsR,
        )
        in_maps.append(imap)

    res = run_bass_kernel_spmd(nc, in_maps, core_ids=list(range(N_CORES)))
    last_exec_time_ns = res.exec_time_ns

    out = np.zeros((T, H), np.float32)
    off = [0, caps[0]]
    for c in range(N_CORES):
        r = res.results[c]
        b = c // 2
        out[b * CS:(b + 1) * CS] += r["sout"].astype(np.float32)
        for el in range(EPC):
            rows, _ = rows_per_e[int(slot_expert[c, el])]
            n = len(rows)
            if n:
                # rows are unique within one expert, so fancy-index add is safe
                out[rows] += r["rout"][off[el]: off[el] + n].astype(np.float32)
    return out.reshape(hs.shape).astype(hs.dtype)
